# revision 31
# baseline (speedup 1.0000x reference)
"""Trainium2 Bass kernel v3 for nn_Block_59433757442280 (spiking local-attention block).

Data-parallel over B=8 (one batch element per core). Single fused pass,
t-interleaved. v3 restructure vs v2:
- No u-evacuation anywhere: spike (s) and decay (m) passes read GEMM PSUM
  directly (DVE/GpSimd tensor_scalar with const scalars).
- Biases enter PSUM via double-fp8 DR matmuls (hi+lo fp8 pair, ~12-bit
  accurate) instead of Act-evac bias / bf16 ones-matmuls.
- proj/f2 LIF use ud8 form: ud = u*[u<TH] in one scalar_tensor_tensor from
  PSUM; feedback matmul uses (ud, zeros) DR pair with [0.5I; 0].
- proj spike fused with residual: x2 = (pf >= TH) + x via stt.
  f2 spike fused with output: out = (pf >= TH) + x2 via stt.
- Attention: two q-blocks share one PSUM bank; one exp per head (no accum),
  row-sums via DVE tensor_reduce; halo transpose/PV restricted to the 8
  first-window columns.
"""

import sys

for _p in ("/opt/trn_rl_repo",):
    if _p not in sys.path:
        sys.path.insert(0, _p)

import numpy as np
import ml_dtypes

import concourse.bass as bass
import concourse.tile as tile
from concourse import mybir, bacc
from concourse.bass_utils import run_bass_kernel_spmd

F32 = mybir.dt.float32
BF16 = mybir.dt.bfloat16
FP8 = mybir.dt.float8e4
AF = mybir.ActivationFunctionType
ALU = mybir.AluOpType
DR = mybir.MatmulPerfMode.DoubleRow
BF = ml_dtypes.bfloat16
E4 = ml_dtypes.float8_e4m3

# problem constants
T, B, NSEQ, C, HD = 4, 8, 1024, 768, 3072
NH, DH, W = 8, 96, 8
TOK = T * NSEQ
CI6 = C // 128            # 6 input-channel tiles
M24 = HD // 128           # 24 f1 output tiles
NCH = 256                 # seq positions per chunk
NCHUNK = NSEQ // NCH      # 4
PB = NCH // 128           # 2 position blocks per chunk
SCALE = float(DH) ** -0.5
SW = 64.0                 # weight scale (power of 2)
TH = 2.0 * SW             # LIF threshold in u domain (vth=1)
THA = 1.0                 # attn-lif threshold (vth=0.5, unscaled)
NEG = -240.0              # fp8 mask value


def build_nc():
    nc = bacc.Bacc(None, target_bir_lowering=False, debug=False)

    # ---- DRAM inputs (per core) ----
    x8_d = nc.dram_tensor("x8", [128, CI6, TOK], FP8, kind="ExternalInput")
    xbf_d = nc.dram_tensor("xbf", [128, CI6, TOK], BF16, kind="ExternalInput")
    wq8_d = nc.dram_tensor("wq8", [128, CI6 * C], FP8, kind="ExternalInput")
    wk8_d = nc.dram_tensor("wk8", [128, CI6 * C], FP8, kind="ExternalInput")
    wv8_d = nc.dram_tensor("wv8", [128, CI6 * C], FP8, kind="ExternalInput")
    wp8_d = nc.dram_tensor("wp8", [96, NH * C], FP8, kind="ExternalInput")
    w18_d = nc.dram_tensor("w18", [128, CI6 * HD], FP8, kind="ExternalInput")
    w28_d = nc.dram_tensor("w28", [128, M24 * C], FP8, kind="ExternalInput")
    # double-fp8 bias pairs
    bqp_d = nc.dram_tensor("bqp", [1, 2 * C], FP8, kind="ExternalInput")
    bkp_d = nc.dram_tensor("bkp", [1, 2 * C], FP8, kind="ExternalInput")
    bvp_d = nc.dram_tensor("bvp", [1, 2 * C], FP8, kind="ExternalInput")
    bpp_d = nc.dram_tensor("bpp", [1, 2 * C], FP8, kind="ExternalInput")
    b1p_d = nc.dram_tensor("b1p", [1, 2 * HD], FP8, kind="ExternalInput")
    b2p_d = nc.dram_tensor("b2p", [1, 2 * C], FP8, kind="ExternalInput")
    ones2_d = nc.dram_tensor("ones2", [1, 2 * NCH], FP8, kind="ExternalInput")
    qp_d = nc.dram_tensor("qp", [16, NCH], FP8, kind="ExternalInput")
    kp_d = nc.dram_tensor("kp", [16, NCH], FP8, kind="ExternalInput")
    khp_d = nc.dram_tensor("khp", [16, T * W], FP8, kind="ExternalInput")
    khf_d = nc.dram_tensor("khf", [16, T * W], FP8, kind="ExternalInput")
    i96s_d = nc.dram_tensor("i96s", [96, 2 * 96], FP8, kind="ExternalInput")
    i96a_d = nc.dram_tensor("i96a", [96, 2 * 96], FP8, kind="ExternalInput")
    i128s_d = nc.dram_tensor("i128s", [128, 2 * 128], FP8, kind="ExternalInput")
    idT_d = nc.dram_tensor("idT", [128, 128], FP8, kind="ExternalInput")
    out_d = nc.dram_tensor("outT", [CI6, NCHUNK, T, 128, NCH], F32,
                           kind="ExternalOutput")

    with tile.TileContext(nc) as tc:
        from contextlib import ExitStack
        with ExitStack() as top:
            cpool = top.enter_context(tc.tile_pool(name="const", bufs=1))
            mspool = top.enter_context(tc.tile_pool(name="ms", bufs=1))
            upool = top.enter_context(tc.tile_pool(name="u", bufs=1))
            xpool = top.enter_context(tc.tile_pool(name="x", bufs=2))
            apool = top.enter_context(tc.tile_pool(name="attn", bufs=3))
            opool = top.enter_context(tc.tile_pool(name="of", bufs=2))
            ps_qk = top.enter_context(tc.tile_pool(name="psqk", bufs=2, space="PSUM"))
            ps_v = top.enter_context(tc.tile_pool(name="psv", bufs=1, space="PSUM"))
            ps_sim = top.enter_context(tc.tile_pool(name="pssim", bufs=1, space="PSUM"))
            ps_tp = top.enter_context(tc.tile_pool(name="pstp", bufs=1, space="PSUM"))
            ps_pv = top.enter_context(tc.tile_pool(name="pspv", bufs=1, space="PSUM"))
            ps_f = top.enter_context(tc.tile_pool(name="psf", bufs=2, space="PSUM"))

            # ---- persistent SBUF ----
            wq8_sb = cpool.tile([128, CI6, C], FP8, name="wq8", tag="wq8")
            nc.sync.dma_start(wq8_sb[:], wq8_d.rearrange("p (a b) -> p a b", a=CI6))
            wk8_sb = cpool.tile([128, CI6, C], FP8, name="wk8", tag="wk8")
            nc.gpsimd.dma_start(wk8_sb[:], wk8_d.rearrange("p (a b) -> p a b", a=CI6))
            wv8_sb = cpool.tile([128, CI6, C], FP8, name="wv8", tag="wv8")
            nc.gpsimd.dma_start(wv8_sb[:], wv8_d.rearrange("p (a b) -> p a b", a=CI6))
            wp8_sb = cpool.tile([96, NH, C], FP8, name="wp8", tag="wp8")
            nc.scalar.dma_start(wp8_sb[:], wp8_d.rearrange("p (a b) -> p a b", a=NH))
            w18_sb = cpool.tile([128, CI6, HD], FP8, name="w18", tag="w18")
            nc.scalar.dma_start(w18_sb[:], w18_d.rearrange("p (a b) -> p a b", a=CI6))
            w28_sb = cpool.tile([128, M24, C], FP8, name="w28", tag="w28")
            nc.gpsimd.dma_start(w28_sb[:], w28_d.rearrange("p (a b) -> p a b", a=M24))
            bqp = cpool.tile([1, 2, C], FP8, name="bqp", tag="bqp")
            nc.sync.dma_start(bqp[:], bqp_d.rearrange("p (a b) -> p a b", a=2))
            bkp = cpool.tile([1, 2, C], FP8, name="bkp", tag="bkp")
            nc.sync.dma_start(bkp[:], bkp_d.rearrange("p (a b) -> p a b", a=2))
            bvp = cpool.tile([1, 2, C], FP8, name="bvp", tag="bvp")
            nc.sync.dma_start(bvp[:], bvp_d.rearrange("p (a b) -> p a b", a=2))
            bpp = cpool.tile([1, 2, C], FP8, name="bpp", tag="bpp")
            nc.sync.dma_start(bpp[:], bpp_d.rearrange("p (a b) -> p a b", a=2))
            b1p = cpool.tile([1, 2, HD], FP8, name="b1p", tag="b1p")
            nc.sync.dma_start(b1p[:], b1p_d.rearrange("p (a b) -> p a b", a=2))
            b2p = cpool.tile([1, 2, C], FP8, name="b2p", tag="b2p")
            nc.sync.dma_start(b2p[:], b2p_d.rearrange("p (a b) -> p a b", a=2))
            ones2 = cpool.tile([1, 2, NCH], FP8, name="ones2", tag="ones2")
            nc.sync.dma_start(ones2[:], ones2_d.rearrange("p (a b) -> p a b", a=2))
            onesP = cpool.tile([1, 2, 128], FP8, name="onesP", tag="onesP")
            nc.sync.dma_start(onesP[:], ones2_d.rearrange("p (a b) -> p a b",
                                                          a=2)[:, :, 0:128])
            i96s_sb = cpool.tile([96, 2, 96], FP8, name="i96s", tag="i96s")
            nc.sync.dma_start(i96s_sb[:], i96s_d.rearrange("p (a b) -> p a b", a=2))
            i96a_sb = cpool.tile([96, 2, 96], FP8, name="i96a", tag="i96a")
            nc.sync.dma_start(i96a_sb[:], i96a_d.rearrange("p (a b) -> p a b", a=2))
            i128s_sb = cpool.tile([128, 2, 128], FP8, name="i128s", tag="i128s")
            nc.sync.dma_start(i128s_sb[:], i128s_d.rearrange("p (a b) -> p a b", a=2))
            idT_sb = cpool.tile([128, 128], FP8, name="idT", tag="idT")
            nc.sync.dma_start(idT_sb[:], idT_d[:])

            # k-halo tiles: [112, NH, T, W]; pattern rows 96:112 loaded once
            khc = cpool.tile([112, NH, T, W], FP8, name="khc", tag="khc")
            khpv = cpool.tile([112, NH, T, W], FP8, name="khpv", tag="khpv")
            khf = cpool.tile([112, NH, T, W], FP8, name="khf", tag="khf")
            for h in range(NH):
                nc.gpsimd.dma_start(khc[96:112, h, :, :],
                                    khp_d.rearrange("g (t w) -> g t w", t=T))
                nc.gpsimd.dma_start(khpv[96:112, h, :, :],
                                    khp_d.rearrange("g (t w) -> g t w", t=T))
                nc.gpsimd.dma_start(khf[96:112, h, :, :],
                                    khf_d.rearrange("g (t w) -> g t w", t=T))
            nc.vector.memset(khf[0:96, :, :, :], 0.0)
            nc.vector.memset(khpv[0:96, :, :, :], 0.0)
            # v-halo: prev-chunk [8, T, C]; cur [8, C] rotating
            vhp_sb = cpool.tile([8, T, C], FP8, name="vhp", tag="vhp")
            nc.vector.memset(vhp_sb[:], 0.0)

            msqk_t = {}
            for nm in ("q", "k"):
                msc = mspool.tile([112, 2, NH, NCH], FP8, name=f"ms{nm}",
                                  tag=f"ms{nm}")
                pat = qp_d if nm == "q" else kp_d
                for h in range(NH):
                    nc.gpsimd.dma_start(msc[96:112, 1, h, :], pat[:])
                msqk_t[nm] = msc

            # persistent m/s tiles (rewritten each t; WAR deps serialize)
            msv = mspool.tile([128, 2, PB, C], FP8, name="msv", tag="msv")
            msoa = mspool.tile([96, 2, NH, NCH], FP8, name="msoa", tag="msoa")
            hm = mspool.tile([128, 2, M24, NCH], FP8, name="hm", tag="hm")
            mso = mspool.tile([128, 2, CI6, NCH], FP8, name="mso", tag="mso")
            msm = mspool.tile([128, 2, CI6, NCH], FP8, name="msm", tag="msm")

            ms_prev = {}

            for c in range(NCHUNK):
                x8c = xpool.tile([128, CI6, NSEQ], FP8, name="x8c", tag="x8c")
                nc.sync.dma_start(x8c[:], x8_d[:, :, c * NSEQ:(c + 1) * NSEQ])
                xbfc = xpool.tile([128, CI6, NSEQ], BF16, name="xbfc", tag="xbfc")
                nc.scalar.dma_start(xbfc[:], xbf_d[:, :, c * NSEQ:(c + 1) * NSEQ])
                for t in range(T):
                    col0 = t * NCH
                    xcols = slice(col0, col0 + NCH)

                    # ========== q, k GEMMs + LIF ==========
                    for nm, w8, bp in (("q", wq8_sb, bqp), ("k", wk8_sb, bkp)):
                        msc = msqk_t[nm]
                        msp = ms_prev.get(nm)
                        for j in range(4):
                            ps = ps_qk.tile([96, 2, NCH], F32, name="psqk",
                                            tag="psqk")
                            for half in range(2):
                                h = 2 * j + half
                                out = ps[:, half, :]
                                for p in range(3):
                                    nc.tensor.matmul(
                                        out,
                                        w8[:, 2 * p:2 * p + 2, h * DH:(h + 1) * DH],
                                        x8c[:, 2 * p:2 * p + 2, xcols],
                                        start=(half == 0 and p == 0), stop=False,
                                        perf_mode=DR)
                                nc.tensor.matmul(
                                    out, bp[0:1, :, h * DH:(h + 1) * DH],
                                    ones2[0:1, :, :], start=False,
                                    stop=(t == 0 and half == 1), perf_mode=DR)
                            if t > 0:
                                nc.tensor.matmul(
                                    ps[:, :, :], i96s_sb[:],
                                    msp[0:96, :, 2 * j:2 * j + 2, :],
                                    start=False, stop=True, perf_mode=DR)
                            nc.vector.tensor_scalar(
                                msc[0:96, 1, 2 * j:2 * j + 2, :], ps[:], TH, None,
                                ALU.is_ge)
                            if t < T - 1:
                                nc.vector.tensor_scalar(
                                    msc[0:96, 0, 2 * j:2 * j + 2, :], ps[:], TH,
                                    0.5, ALU.min, ALU.mult)
                        ms_prev[nm] = msc
                    msq, msk = msqk_t["q"], msqk_t["k"]

                    # k halos: within-chunk (cols 120:128) for qb=1
                    nc.vector.tensor_copy(khc[0:96, :, t, :],
                                          msk[0:96, 1, :, 120:128])

                    # ========== v GEMM + LIF ==========
                    mspv = ms_prev.get("v")
                    for pb in range(PB):
                        pcol = col0 + pb * 128
                        for half in range(2):
                            ps = ps_v.tile([128, 384], F32, name="psv", tag="psv")
                            for p in range(3):
                                nc.tensor.matmul(
                                    ps[:], x8c[:, 2 * p:2 * p + 2, pcol:pcol + 128],
                                    wv8_sb[:, 2 * p:2 * p + 2,
                                           half * 384:(half + 1) * 384],
                                    start=(p == 0), stop=False, perf_mode=DR)
                            nc.tensor.matmul(
                                ps[:], onesP[0:1, :, :],
                                bvp[0:1, :, half * 384:(half + 1) * 384],
                                start=False, stop=(t == 0), perf_mode=DR)
                            if t > 0:
                                nc.tensor.matmul(
                                    ps[:], i128s_sb[:],
                                    mspv[:, :, pb, half * 384:(half + 1) * 384],
                                    start=False, stop=True, perf_mode=DR)
                            nc.vector.tensor_scalar(
                                msv[:, 1, pb, half * 384:(half + 1) * 384], ps[:],
                                TH, None, ALU.is_ge)
                            if t < T - 1:
                                nc.vector.tensor_scalar(
                                    msv[:, 0, pb, half * 384:(half + 1) * 384],
                                    ps[:], TH, 0.5, ALU.min, ALU.mult)
                    ms_prev["v"] = msv
                    # v halo for within-chunk qb=1 (pb0 tail)
                    vhc = apool.tile([8, C], FP8, name="vhc", tag="vhc")
                    nc.sync.dma_start(vhc[:], msv[120:128, 1, 0, :])

                    # ========== attention ==========
                    msop = ms_prev.get("oa")
                    for j in range(4):
                        ppv = ps_pv.tile([96, 2, NCH], F32, name="pspv", tag="pspv")
                        for half in range(2):
                            h = 2 * j + half
                            psm = ps_sim.tile([128, 2, 136], F32, name="pssim",
                                              tag="pssim")
                            for qb in range(2):
                                qsl = msq[0:112, 1, h, qb * 128:(qb + 1) * 128]
                                nc.tensor.matmul(
                                    psm[:, qb, 0:128], qsl,
                                    msk[0:112, 1, h, qb * 128:(qb + 1) * 128],
                                    start=(qb == 0), stop=False)
                                halo = (khf if (c == 0 and qb == 0)
                                        else khpv if qb == 0 else khc)
                                nc.tensor.matmul(psm[:, qb, 128:136], qsl,
                                                 halo[0:112, h, t, :],
                                                 start=False, stop=(qb == 1))
                            attn = apool.tile([128, 2, 136], BF16, name="attn",
                                              tag="attn")
                            nc.scalar.activation(attn[:], psm[:], AF.Exp,
                                                 scale=SCALE)
                            rs = apool.tile([128, 2], F32, name="rs", tag="rs")
                            nc.vector.tensor_reduce(rs[:], attn[:],
                                                    mybir.AxisListType.X, ALU.add)
                            rc = apool.tile([128, 2], F32, name="rc", tag="rc")
                            nc.vector.reciprocal(rc[:], rs[:])
                            for qb in range(2):
                                at8 = apool.tile([128, 136], FP8, name="at8",
                                                 tag="at8")
                                nc.vector.tensor_scalar(at8[:], attn[:, qb, :],
                                                        rc[:, qb:qb + 1], None,
                                                        ALU.mult)
                                tpm = ps_tp.tile([128, 288], FP8, name="tpm",
                                                 tag="tp")
                                nc.tensor.matmul(tpm[:, 0:256:2], at8[:, 0:128],
                                                 idT_sb[:], start=True, stop=True,
                                                 is_transpose=True)
                                nc.tensor.matmul(tpm[0:8, 272:288:2],
                                                 at8[0:8, 128:136],
                                                 idT_sb[0:8, 0:8], start=False,
                                                 stop=False, is_transpose=True,
                                                 skip_group_check=True)
                                am = apool.tile([128, 128], FP8, name="am", tag="am")
                                nc.vector.tensor_copy(am[:, :], tpm[:, 0:256:2])
                                amh = apool.tile([8, 8], FP8, name="amh", tag="amh")
                                nc.vector.tensor_copy(amh[:, :], tpm[0:8, 272:288:2])
                                out = ppv[:, half, qb * 128:(qb + 1) * 128]
                                nc.tensor.matmul(
                                    out, msv[:, 1, qb, h * DH:(h + 1) * DH],
                                    am[:, :],
                                    start=(half == 0 and qb == 0), stop=False)
                                outh = ppv[:, half, qb * 128:qb * 128 + 8]
                                vhalo = (vhp_sb[0:8, t, h * DH:(h + 1) * DH]
                                         if qb == 0
                                         else vhc[0:8, h * DH:(h + 1) * DH])
                                nc.tensor.matmul(
                                    outh, vhalo, amh[:, :], start=False,
                                    stop=(t == 0 and half == 1 and qb == 1))
                        if t > 0:
                            nc.tensor.matmul(ppv[:, :, :], i96a_sb[:],
                                             msop[0:96, :, 2 * j:2 * j + 2, :],
                                             start=False, stop=True, perf_mode=DR)
                        nc.vector.tensor_scalar(
                            msoa[0:96, 1, 2 * j:2 * j + 2, :], ppv[:], THA, None,
                            ALU.is_ge)
                        if t < T - 1:
                            nc.vector.tensor_scalar(
                                msoa[0:96, 0, 2 * j:2 * j + 2, :], ppv[:], THA,
                                0.5, ALU.min, ALU.mult)
                    ms_prev["oa"] = msoa

                    # halo captures for next chunk (after attention reads)
                    nc.vector.tensor_copy(khpv[0:96, :, t, :],
                                          msk[0:96, 1, :, NCH - 8:NCH])
                    nc.sync.dma_start(vhp_sb[0:8, t, :], msv[120:128, 1, 1, :])

                    # ========== proj (ud8 LIF) + x2 = x + o ==========
                    x2 = upool.tile([128, CI6, NCH], BF16, name="x2", tag="x2")
                    for jj in range(3):
                        pf = ps_f.tile([128, 2, NCH], F32, name="psf", tag="psf")
                        for half in range(2):
                            i = 2 * jj + half
                            out = pf[:, half, :]
                            for hp in range(4):
                                nc.tensor.matmul(
                                    out,
                                    wp8_sb[:, 2 * hp:2 * hp + 2,
                                           i * 128:(i + 1) * 128],
                                    msoa[0:96, 1, 2 * hp:2 * hp + 2, :],
                                    start=(half == 0 and hp == 0), stop=False,
                                    perf_mode=DR)
                            nc.tensor.matmul(
                                out, bpp[0:1, :, i * 128:(i + 1) * 128],
                                ones2[0:1, :, :], start=False,
                                stop=(t == 0 and half == 1), perf_mode=DR)
                        if t > 0:
                            nc.tensor.matmul(
                                pf[:, :, :], i128s_sb[:],
                                mso[:, :, 2 * jj:2 * jj + 2, :],
                                start=False, stop=True, perf_mode=DR)
                        nc.vector.tensor_scalar(
                            mso[:, 1, 2 * jj:2 * jj + 2, :], pf[:], TH, None,
                            ALU.is_ge)
                        if t < T - 1:
                            nc.vector.tensor_scalar(
                                mso[:, 0, 2 * jj:2 * jj + 2, :], pf[:], TH,
                                0.5, ALU.min, ALU.mult)
                    nc.gpsimd.tensor_tensor(x2[:], mso[:, 1, :, :],
                                            xbfc[:, :, xcols], ALU.add)
                    x28 = upool.tile([128, CI6, NCH], FP8, name="x28", tag="x28")
                    nc.gpsimd.tensor_copy(x28[:], x2[:])

                    # ========== f1 (ms LIF, plane-major hm) ==========
                    hmp = ms_prev.get("h")
                    for jj in range(12):
                        pf = ps_f.tile([128, 2, NCH], F32, name="psf", tag="psf")
                        for half in range(2):
                            i = 2 * jj + half
                            out = pf[:, half, :]
                            for p in range(3):
                                nc.tensor.matmul(
                                    out,
                                    w18_sb[:, 2 * p:2 * p + 2,
                                           i * 128:(i + 1) * 128],
                                    x28[:, 2 * p:2 * p + 2, :],
                                    start=(half == 0 and p == 0), stop=False,
                                    perf_mode=DR)
                            nc.tensor.matmul(
                                out, b1p[0:1, :, i * 128:(i + 1) * 128],
                                ones2[0:1, :, :], start=False,
                                stop=(t == 0 and half == 1), perf_mode=DR)
                        if t > 0:
                            nc.tensor.matmul(
                                pf[:, :, :], i128s_sb[:],
                                hmp[:, :, 2 * jj:2 * jj + 2, :],
                                start=False, stop=True, perf_mode=DR)
                        nc.vector.tensor_scalar(
                            hm[:, 1, 2 * jj:2 * jj + 2, :], pf[:], TH, None,
                            ALU.is_ge)
                        if t < T - 1:
                            nc.vector.tensor_scalar(
                                hm[:, 0, 2 * jj:2 * jj + 2, :], pf[:], TH, 0.5,
                                ALU.min, ALU.mult)
                    ms_prev["h"] = hm

                    # ========== f2 (ud8 LIF) + out = x2 + m ==========
                    of = opool.tile([128, CI6, NCH], F32, name="of", tag="of")
                    for jj in range(3):
                        pf = ps_f.tile([128, 2, NCH], F32, name="psf", tag="psf")
                        for half in range(2):
                            i = 2 * jj + half
                            out = pf[:, half, :]
                            for p in range(12):
                                nc.tensor.matmul(
                                    out,
                                    w28_sb[:, 2 * p:2 * p + 2,
                                           i * 128:(i + 1) * 128],
                                    hm[:, 1, 2 * p:2 * p + 2, :],
                                    start=(half == 0 and p == 0), stop=False,
                                    perf_mode=DR)
                            nc.tensor.matmul(
                                out, b2p[0:1, :, i * 128:(i + 1) * 128],
                                ones2[0:1, :, :], start=False,
                                stop=(t == 0 and half == 1), perf_mode=DR)
                        if t > 0:
                            nc.tensor.matmul(
                                pf[:, :, :], i128s_sb[:],
                                msm[:, :, 2 * jj:2 * jj + 2, :],
                                start=False, stop=True, perf_mode=DR)
                        nc.vector.tensor_scalar(
                            msm[:, 1, 2 * jj:2 * jj + 2, :], pf[:], TH, None,
                            ALU.is_ge)
                        if t < T - 1:
                            nc.vector.tensor_scalar(
                                msm[:, 0, 2 * jj:2 * jj + 2, :], pf[:], TH,
                                0.5, ALU.min, ALU.mult)
                    nc.gpsimd.tensor_tensor(of[:], msm[:, 1, :, :], x2[:], ALU.add)
                    nc.sync.dma_start(
                        out_d[:, c, t].rearrange("a p n -> p a n"), of[:])

    nc.compile()
    return nc


# ---------------- host-side preparation ----------------

def _lhsT(w, s, nci, npart=128):
    """fold BN scale, scale by SW, fp8, and lay out as [npart, nci, out]"""
    wf = (w * s[:, None]).astype(np.float32) * SW
    out_dim = wf.shape[0]
    return np.ascontiguousarray(
        wf.T.reshape(nci, npart, out_dim).transpose(1, 0, 2)).astype(E4)


def _bias_pair(b):
    """double-fp8 (hi, lo) pair of a bias row, as [1, 2*dim]"""
    bf = np.asarray(b, np.float32)
    hi = bf.astype(E4)
    lo = (bf - hi.astype(np.float32)).astype(E4)
    return np.concatenate([hi[None, :], lo[None, :]], axis=0).reshape(1, -1)


def _prep_shared(qw, qb, qs, qt, kw, kb, ks, kt, vw, vb, vs, vt,
                 pw, pb, ps, pt, f1w, f1b, f1s, f1t, f2w, f2b, f2s, f2t):
    out = {}
    out["wq8"] = _lhsT(qw, qs, CI6).reshape(128, CI6 * C)
    out["wk8"] = _lhsT(kw, ks, CI6).reshape(128, CI6 * C)
    out["wv8"] = _lhsT(vw, vs, CI6).reshape(128, CI6 * C)
    out["wp8"] = _lhsT(pw, ps, NH, 96).reshape(96, NH * C)
    out["w18"] = _lhsT(f1w, f1s, CI6).reshape(128, CI6 * HD)
    out["w28"] = _lhsT(f2w, f2s, M24).reshape(128, M24 * C)
    # biases, reordered to match each GEMM's output tiling
    bq = (qb * qs + qt).astype(np.float32) * SW          # by head already natural
    out["bqp"] = _bias_pair(bq)
    bk = (kb * ks + kt).astype(np.float32) * SW
    out["bkp"] = _bias_pair(bk)
    bv = (vb * vs + vt).astype(np.float32) * SW
    out["bvp"] = _bias_pair(bv)
    bp_ = (pb * ps + pt).astype(np.float32) * SW
    out["bpp"] = _bias_pair(bp_)
    b1 = (f1b * f1s + f1t).astype(np.float32) * SW
    out["b1p"] = _bias_pair(b1)
    b2 = (f2b * f2s + f2t).astype(np.float32) * SW
    out["b2p"] = _bias_pair(b2)
    out["ones2"] = np.ones((1, 2 * NCH), dtype=E4)

    qp = np.zeros((16, NCH), dtype=np.float32)
    kp = np.zeros((16, NCH), dtype=np.float32)
    for col in range(NCH):
        j = col % 128
        qp[j // W, col] = 1.0
        jwin = j + W
        for g in range(16):
            kp[g, col] = 0.0 if (W * g <= jwin < W * g + 2 * W) else NEG
    out["qp"] = qp.astype(E4)
    out["kp"] = kp.astype(E4)
    khp = np.full((16, W), NEG, dtype=np.float32)
    khp[0, :] = 0.0
    out["khp"] = np.tile(khp, (1, T)).astype(E4)
    out["khf"] = np.full((16, T * W), NEG, dtype=E4)

    eye96 = np.eye(96, dtype=np.float32)
    out["i96s"] = np.concatenate([eye96[:, None, :], -(TH / 2) * eye96[:, None, :]],
                                 axis=1).reshape(96, 2 * 96).astype(E4)
    out["i96a"] = np.concatenate([eye96[:, None, :], -0.5 * eye96[:, None, :]],
                                 axis=1).reshape(96, 2 * 96).astype(E4)
    eye128 = np.eye(128, dtype=np.float32)
    out["i128s"] = np.concatenate([eye128[:, None, :], -(TH / 2) * eye128[:, None, :]],
                                  axis=1).reshape(128, 2 * 128).astype(E4)
    out["idT"] = np.eye(128, dtype=E4)
    return out


def prep_in_maps(inputs):
    x = np.asarray(inputs["x"], dtype=np.float32)
    shared = _prep_shared(**{k: np.asarray(v, np.float32)
                             for k, v in inputs.items() if k != "x"})
    in_maps = []
    for b in range(B):
        xb = x[:, b]                                    # [T, N, C]
        y = np.ascontiguousarray(xb.transpose(2, 0, 1)) # [C, T, N]
        y = y.reshape(CI6, 128, T, NCHUNK, NCH)
        arr = np.ascontiguousarray(y.transpose(1, 0, 3, 2, 4)).reshape(128, CI6, TOK)
        m = dict(shared)
        m["xbf"] = arr.astype(BF)
        m["x8"] = arr.astype(E4)
        in_maps.append(m)
    return in_maps


_NC_CACHE = {}


def get_nc():
    if "nc" not in _NC_CACHE:
        _NC_CACHE["nc"] = build_nc()
    return _NC_CACHE["nc"]


def assemble_output(results):
    out = np.empty((T, B, NSEQ, C), dtype=np.float32)
    for b in range(B):
        arr = results[b]["outT"]                        # [CI6, NCHUNK, T, 128, NCH]
        out[:, b] = arr.transpose(2, 1, 4, 0, 3).reshape(T, NSEQ, C)
    return out


def kernel(**inputs):
    nc = get_nc()
    in_maps = prep_in_maps(inputs)
    res = run_bass_kernel_spmd(nc, in_maps, list(range(B)))
    return assemble_output(res.results)


if __name__ == "__main__":
    nc = get_nc()
    print("compiled OK")


# revision 40
# speedup vs baseline: 1.6529x; 1.6529x over previous
"""Trainium2 Bass kernel v3 for nn_Block_59433757442280 (spiking local-attention block).

Data-parallel over B=8 (one batch element per core). Single fused pass,
t-interleaved. v3 restructure vs v2:
- No u-evacuation anywhere: spike (s) and decay (m) passes read GEMM PSUM
  directly (DVE/GpSimd tensor_scalar with const scalars).
- Biases enter PSUM via double-fp8 DR matmuls (hi+lo fp8 pair, ~12-bit
  accurate) instead of Act-evac bias / bf16 ones-matmuls.
- proj/f2 LIF use ud8 form: ud = u*[u<TH] in one scalar_tensor_tensor from
  PSUM; feedback matmul uses (ud, zeros) DR pair with [0.5I; 0].
- proj spike fused with residual: x2 = (pf >= TH) + x via stt.
  f2 spike fused with output: out = (pf >= TH) + x2 via stt.
- Attention: two q-blocks share one PSUM bank; one exp per head (no accum),
  row-sums via DVE tensor_reduce; halo transpose/PV restricted to the 8
  first-window columns.
"""

import sys

for _p in ("/opt/trn_rl_repo",):
    if _p not in sys.path:
        sys.path.insert(0, _p)

import numpy as np
import ml_dtypes

import concourse.bass as bass
import concourse.tile as tile
from concourse import mybir, bacc
from concourse.bass_utils import run_bass_kernel_spmd

F32 = mybir.dt.float32
BF16 = mybir.dt.bfloat16
FP8 = mybir.dt.float8e4
AF = mybir.ActivationFunctionType
ALU = mybir.AluOpType
DR = mybir.MatmulPerfMode.DoubleRow
BF = ml_dtypes.bfloat16
E4 = ml_dtypes.float8_e4m3

# problem constants
T, B, NSEQ, C, HD = 4, 8, 1024, 768, 3072
NH, DH, W = 8, 96, 8
TOK = T * NSEQ
CI6 = C // 128            # 6 input-channel tiles
M24 = HD // 128           # 24 f1 output tiles
NCH = 256                 # seq positions per chunk
NCHUNK = NSEQ // NCH      # 4
PB = NCH // 128           # 2 position blocks per chunk
SCALE = float(DH) ** -0.5
SW = 64.0                 # weight scale (power of 2)
TH = 2.0 * SW             # LIF threshold in u domain (vth=1)
THA = 1.0                 # attn-lif threshold (vth=0.5, unscaled)
NEG = -240.0              # fp8 mask value


def build_nc():
    nc = bacc.Bacc(None, target_bir_lowering=False, debug=False)

    # ---- DRAM inputs (per core) ----
    x8_d = nc.dram_tensor("x8", [128, CI6, TOK], FP8, kind="ExternalInput")
    xbf_d = nc.dram_tensor("xbf", [128, CI6, TOK], BF16, kind="ExternalInput")
    wq8_d = nc.dram_tensor("wq8", [128, CI6 * C], FP8, kind="ExternalInput")
    wk8_d = nc.dram_tensor("wk8", [128, CI6 * C], FP8, kind="ExternalInput")
    wv8_d = nc.dram_tensor("wv8", [128, CI6 * C], FP8, kind="ExternalInput")
    wp8_d = nc.dram_tensor("wp8", [96, NH * C], FP8, kind="ExternalInput")
    w18_d = nc.dram_tensor("w18", [128, CI6 * HD], FP8, kind="ExternalInput")
    w28_d = nc.dram_tensor("w28", [128, M24 * C], FP8, kind="ExternalInput")
    # double-fp8 bias pairs
    bqp_d = nc.dram_tensor("bqp", [1, 2 * C], FP8, kind="ExternalInput")
    bkp_d = nc.dram_tensor("bkp", [1, 2 * C], FP8, kind="ExternalInput")
    bvp_d = nc.dram_tensor("bvp", [1, 2 * C], FP8, kind="ExternalInput")
    bpp_d = nc.dram_tensor("bpp", [1, 2 * C], FP8, kind="ExternalInput")
    b1p_d = nc.dram_tensor("b1p", [1, 2 * HD], FP8, kind="ExternalInput")
    b2p_d = nc.dram_tensor("b2p", [1, 2 * C], FP8, kind="ExternalInput")
    ones2_d = nc.dram_tensor("ones2", [1, 2 * NCH], FP8, kind="ExternalInput")
    qp_d = nc.dram_tensor("qp", [16, NCH], FP8, kind="ExternalInput")
    kp_d = nc.dram_tensor("kp", [16, NCH], FP8, kind="ExternalInput")
    khp_d = nc.dram_tensor("khp", [16, T * W], FP8, kind="ExternalInput")
    khf_d = nc.dram_tensor("khf", [16, T * W], FP8, kind="ExternalInput")
    i96r_d = nc.dram_tensor("i96r", [96, 2 * 96], FP8, kind="ExternalInput")
    i96ar_d = nc.dram_tensor("i96ar", [96, 2 * 96], FP8, kind="ExternalInput")
    i128r_d = nc.dram_tensor("i128r", [128, 2 * 128], FP8, kind="ExternalInput")
    i128s_d = nc.dram_tensor("i128s", [128, 2 * 128], FP8, kind="ExternalInput")
    idT_d = nc.dram_tensor("idT", [128, 128], FP8, kind="ExternalInput")
    out_d = nc.dram_tensor("outT", [CI6, NCHUNK, T, 128, NCH], F32,
                           kind="ExternalOutput")

    with tile.TileContext(nc) as tc:
        from contextlib import ExitStack
        with ExitStack() as top:
            cpool = top.enter_context(tc.tile_pool(name="const", bufs=1))
            mspool = top.enter_context(tc.tile_pool(name="ms", bufs=1))
            upool = top.enter_context(tc.tile_pool(name="u", bufs=1))
            xpool = top.enter_context(tc.tile_pool(name="x", bufs=2))
            apool = top.enter_context(tc.tile_pool(name="attn", bufs=3))
            opool = top.enter_context(tc.tile_pool(name="of", bufs=2))
            ps_qk = top.enter_context(tc.tile_pool(name="psqk", bufs=2, space="PSUM"))
            ps_v = top.enter_context(tc.tile_pool(name="psv", bufs=1, space="PSUM"))
            ps_sim = top.enter_context(tc.tile_pool(name="pssim", bufs=1, space="PSUM"))
            ps_tp = top.enter_context(tc.tile_pool(name="pstp", bufs=1, space="PSUM"))
            ps_pv = top.enter_context(tc.tile_pool(name="pspv", bufs=1, space="PSUM"))
            ps_f = top.enter_context(tc.tile_pool(name="psf", bufs=2, space="PSUM"))

            # ---- persistent SBUF ----
            wq8_sb = cpool.tile([128, CI6, C], FP8, name="wq8", tag="wq8")
            nc.sync.dma_start(wq8_sb[:], wq8_d.rearrange("p (a b) -> p a b", a=CI6))
            wk8_sb = cpool.tile([128, CI6, C], FP8, name="wk8", tag="wk8")
            nc.gpsimd.dma_start(wk8_sb[:], wk8_d.rearrange("p (a b) -> p a b", a=CI6))
            wv8_sb = cpool.tile([128, CI6, C], FP8, name="wv8", tag="wv8")
            nc.gpsimd.dma_start(wv8_sb[:], wv8_d.rearrange("p (a b) -> p a b", a=CI6))
            wp8_sb = cpool.tile([96, NH, C], FP8, name="wp8", tag="wp8")
            nc.scalar.dma_start(wp8_sb[:], wp8_d.rearrange("p (a b) -> p a b", a=NH))
            w18_sb = cpool.tile([128, CI6, HD], FP8, name="w18", tag="w18")
            nc.scalar.dma_start(w18_sb[:], w18_d.rearrange("p (a b) -> p a b", a=CI6))
            w28_sb = cpool.tile([128, M24, C], FP8, name="w28", tag="w28")
            nc.gpsimd.dma_start(w28_sb[:], w28_d.rearrange("p (a b) -> p a b", a=M24))
            bqp = cpool.tile([1, 2, C], FP8, name="bqp", tag="bqp")
            nc.sync.dma_start(bqp[:], bqp_d.rearrange("p (a b) -> p a b", a=2))
            bkp = cpool.tile([1, 2, C], FP8, name="bkp", tag="bkp")
            nc.sync.dma_start(bkp[:], bkp_d.rearrange("p (a b) -> p a b", a=2))
            bvp = cpool.tile([1, 2, C], FP8, name="bvp", tag="bvp")
            nc.sync.dma_start(bvp[:], bvp_d.rearrange("p (a b) -> p a b", a=2))
            bpp = cpool.tile([1, 2, C], FP8, name="bpp", tag="bpp")
            nc.sync.dma_start(bpp[:], bpp_d.rearrange("p (a b) -> p a b", a=2))
            b1p = cpool.tile([1, 2, HD], FP8, name="b1p", tag="b1p")
            nc.sync.dma_start(b1p[:], b1p_d.rearrange("p (a b) -> p a b", a=2))
            b2p = cpool.tile([1, 2, C], FP8, name="b2p", tag="b2p")
            nc.sync.dma_start(b2p[:], b2p_d.rearrange("p (a b) -> p a b", a=2))
            ones2 = cpool.tile([1, 2, NCH], FP8, name="ones2", tag="ones2")
            nc.sync.dma_start(ones2[:], ones2_d.rearrange("p (a b) -> p a b", a=2))
            onesP = cpool.tile([1, 2, 128], FP8, name="onesP", tag="onesP")
            nc.sync.dma_start(onesP[:], ones2_d.rearrange("p (a b) -> p a b",
                                                          a=2)[:, :, 0:128])
            i96r_sb = cpool.tile([96, 2, 96], FP8, name="i96r", tag="i96r")
            nc.sync.dma_start(i96r_sb[:], i96r_d.rearrange("p (a b) -> p a b", a=2))
            i96ar_sb = cpool.tile([96, 2, 96], FP8, name="i96ar", tag="i96ar")
            nc.sync.dma_start(i96ar_sb[:], i96ar_d.rearrange("p (a b) -> p a b", a=2))
            i128r_sb = cpool.tile([128, 2, 128], FP8, name="i128r", tag="i128r")
            nc.sync.dma_start(i128r_sb[:], i128r_d.rearrange("p (a b) -> p a b", a=2))
            # Act bias const tiles for m~ = relu(thr/2 - 0.5*psum)
            bc64 = cpool.tile([128, 1], F32, name="bc64", tag="bc64")
            nc.vector.memset(bc64[:], 64.0)
            bc32 = cpool.tile([128, 1], F32, name="bc32", tag="bc32")
            nc.vector.memset(bc32[:], 32.0)
            bcA5 = cpool.tile([128, 1], F32, name="bcA5", tag="bcA5")
            nc.vector.memset(bcA5[:], 0.5)
            bcA25 = cpool.tile([128, 1], F32, name="bcA25", tag="bcA25")
            nc.vector.memset(bcA25[:], 0.25)
            i128s_sb = cpool.tile([128, 2, 128], FP8, name="i128s", tag="i128s")
            nc.sync.dma_start(i128s_sb[:], i128s_d.rearrange("p (a b) -> p a b", a=2))
            idT_sb = cpool.tile([128, 128], FP8, name="idT", tag="idT")
            nc.sync.dma_start(idT_sb[:], idT_d[:])

            # k-halo tiles: [112, NH, T, W]; pattern rows 96:112 loaded once
            khc = cpool.tile([112, NH, T, W], FP8, name="khc", tag="khc")
            khpv = cpool.tile([112, NH, T, W], FP8, name="khpv", tag="khpv")
            khf = cpool.tile([112, NH, T, W], FP8, name="khf", tag="khf")
            for h in range(NH):
                nc.gpsimd.dma_start(khc[96:112, h, :, :],
                                    khp_d.rearrange("g (t w) -> g t w", t=T))
                nc.gpsimd.dma_start(khpv[96:112, h, :, :],
                                    khp_d.rearrange("g (t w) -> g t w", t=T))
                nc.gpsimd.dma_start(khf[96:112, h, :, :],
                                    khf_d.rearrange("g (t w) -> g t w", t=T))
            nc.vector.memset(khf[0:96, :, :, :], 0.0)
            nc.vector.memset(khpv[0:96, :, :, :], 0.0)
            # v-halo: prev-chunk [8, T, C]; cur [8, C] rotating
            vhp_sb = cpool.tile([8, T, C], FP8, name="vhp", tag="vhp")
            nc.vector.memset(vhp_sb[:], 0.0)

            msqk_t = {}
            for nm in ("q", "k"):
                msc = mspool.tile([112, 2, NH, NCH], FP8, name=f"ms{nm}",
                                  tag=f"ms{nm}")
                pat = qp_d if nm == "q" else kp_d
                for h in range(NH):
                    nc.gpsimd.dma_start(msc[96:112, 1, h, :], pat[:])
                msqk_t[nm] = msc

            # persistent m/s tiles (rewritten each t; WAR deps serialize)
            msv = mspool.tile([128, 2, PB, C], FP8, name="msv", tag="msv")
            msoa = mspool.tile([96, 2, NH, NCH], FP8, name="msoa", tag="msoa")
            hm = mspool.tile([128, 2, M24, NCH], FP8, name="hm", tag="hm")
            mso = mspool.tile([128, 2, CI6, NCH], FP8, name="mso", tag="mso")
            msm = mspool.tile([128, 2, CI6, NCH], FP8, name="msm", tag="msm")

            ms_prev = {}

            for c in range(NCHUNK):
                x8c = xpool.tile([128, CI6, NSEQ], FP8, name="x8c", tag="x8c")
                nc.sync.dma_start(x8c[:], x8_d[:, :, c * NSEQ:(c + 1) * NSEQ])
                xbfc = xpool.tile([128, CI6, NSEQ], BF16, name="xbfc", tag="xbfc")
                nc.scalar.dma_start(xbfc[:], xbf_d[:, :, c * NSEQ:(c + 1) * NSEQ])
                for t in range(T):
                    col0 = t * NCH
                    xcols = slice(col0, col0 + NCH)

                    # ========== q, k GEMMs + LIF ==========
                    for nm, w8, bp in (("q", wq8_sb, bqp), ("k", wk8_sb, bkp)):
                        msc = msqk_t[nm]
                        msp = ms_prev.get(nm)
                        for j in range(4):
                            ps = ps_qk.tile([96, 2, NCH], F32, name="psqk",
                                            tag="psqk")
                            for half in range(2):
                                h = 2 * j + half
                                out = ps[:, half, :]
                                for p in range(3):
                                    nc.tensor.matmul(
                                        out,
                                        w8[:, 2 * p:2 * p + 2, h * DH:(h + 1) * DH],
                                        x8c[:, 2 * p:2 * p + 2, xcols],
                                        start=(half == 0 and p == 0), stop=False,
                                        perf_mode=DR)
                                nc.tensor.matmul(
                                    out, bp[0:1, :, h * DH:(h + 1) * DH],
                                    ones2[0:1, :, :], start=False,
                                    stop=(t == 0 and half == 1), perf_mode=DR)
                            if t > 0:
                                nc.tensor.matmul(
                                    ps[:, :, :], i96r_sb[:],
                                    msp[0:96, :, 2 * j:2 * j + 2, :],
                                    start=False, stop=True, perf_mode=DR)
                            nc.vector.tensor_scalar(
                                msc[0:96, 1, 2 * j:2 * j + 2, :], ps[:],
                                TH if t == 0 else TH / 2, None, ALU.is_ge)
                            if t < T - 1:
                                nc.scalar.activation(
                                    msc[0:96, 0, 2 * j:2 * j + 2, :], ps[:],
                                    AF.Relu, scale=-0.5,
                                    bias=(bc64 if t == 0 else bc32)[0:96, :])
                        ms_prev[nm] = msc
                    msq, msk = msqk_t["q"], msqk_t["k"]

                    # k halos: within-chunk (cols 120:128) for qb=1
                    nc.gpsimd.tensor_copy(khc[0:96, :, t, :],
                                          msk[0:96, 1, :, 120:128])

                    # ========== v GEMM + LIF ==========
                    mspv = ms_prev.get("v")
                    for pb in range(PB):
                        pcol = col0 + pb * 128
                        for half in range(2):
                            ps = ps_v.tile([128, 384], F32, name="psv", tag="psv")
                            for p in range(3):
                                nc.tensor.matmul(
                                    ps[:], x8c[:, 2 * p:2 * p + 2, pcol:pcol + 128],
                                    wv8_sb[:, 2 * p:2 * p + 2,
                                           half * 384:(half + 1) * 384],
                                    start=(p == 0), stop=False, perf_mode=DR)
                            nc.tensor.matmul(
                                ps[:], onesP[0:1, :, :],
                                bvp[0:1, :, half * 384:(half + 1) * 384],
                                start=False, stop=(t == 0), perf_mode=DR)
                            if t > 0:
                                nc.tensor.matmul(
                                    ps[:], i128r_sb[:],
                                    mspv[:, :, pb, half * 384:(half + 1) * 384],
                                    start=False, stop=True, perf_mode=DR)
                            nc.vector.tensor_scalar(
                                msv[:, 1, pb, half * 384:(half + 1) * 384], ps[:],
                                TH if t == 0 else TH / 2, None, ALU.is_ge)
                            if t < T - 1:
                                nc.scalar.activation(
                                    msv[:, 0, pb, half * 384:(half + 1) * 384],
                                    ps[:], AF.Relu, scale=-0.5,
                                    bias=bc64 if t == 0 else bc32)
                    ms_prev["v"] = msv
                    # v halo for within-chunk qb=1 (pb0 tail)
                    vhc = apool.tile([8, C], FP8, name="vhc", tag="vhc")
                    nc.sync.dma_start(vhc[:], msv[120:128, 1, 0, :])

                    # ========== attention ==========
                    msop = ms_prev.get("oa")
                    for j in range(4):
                        ppv = ps_pv.tile([96, 2, NCH], F32, name="pspv", tag="pspv")
                        for half in range(2):
                            h = 2 * j + half
                            psm = ps_sim.tile([128, 2, 136], F32, name="pssim",
                                              tag="pssim")
                            for qb in range(2):
                                qsl = msq[0:112, 1, h, qb * 128:(qb + 1) * 128]
                                nc.tensor.matmul(
                                    psm[:, qb, 0:128], qsl,
                                    msk[0:112, 1, h, qb * 128:(qb + 1) * 128],
                                    start=(qb == 0), stop=False)
                                halo = (khf if (c == 0 and qb == 0)
                                        else khpv if qb == 0 else khc)
                                nc.tensor.matmul(psm[:, qb, 128:136], qsl,
                                                 halo[0:112, h, t, :],
                                                 start=False, stop=(qb == 1))
                            attn = apool.tile([128, 2, 136], BF16, name="attn",
                                              tag="attn")
                            nc.scalar.activation(attn[:], psm[:], AF.Exp,
                                                 scale=SCALE)
                            rs = apool.tile([128, 2], F32, name="rs", tag="rs")
                            nc.vector.tensor_reduce(rs[:], attn[:],
                                                    mybir.AxisListType.X, ALU.add)
                            rc = apool.tile([128, 2], F32, name="rc", tag="rc")
                            nc.vector.reciprocal(rc[:], rs[:])
                            for qb in range(2):
                                at8 = apool.tile([128, 136], FP8, name="at8",
                                                 tag="at8")
                                nc.vector.tensor_scalar(at8[:], attn[:, qb, :],
                                                        rc[:, qb:qb + 1], None,
                                                        ALU.mult)
                                tpm = ps_tp.tile([128, 288], FP8, name="tpm",
                                                 tag="tp")
                                nc.tensor.matmul(tpm[:, 0:256:2], at8[:, 0:128],
                                                 idT_sb[:], start=True, stop=True,
                                                 is_transpose=True)
                                nc.tensor.matmul(tpm[0:8, 272:288:2],
                                                 at8[0:8, 128:136],
                                                 idT_sb[0:8, 0:8], start=False,
                                                 stop=False, is_transpose=True,
                                                 skip_group_check=True)
                                am = apool.tile([128, 128], FP8, name="am", tag="am")
                                nc.vector.tensor_copy(am[:, :], tpm[:, 0:256:2])
                                amh = apool.tile([8, 8], FP8, name="amh", tag="amh")
                                nc.vector.tensor_copy(amh[:, :], tpm[0:8, 272:288:2])
                                out = ppv[:, half, qb * 128:(qb + 1) * 128]
                                nc.tensor.matmul(
                                    out, msv[:, 1, qb, h * DH:(h + 1) * DH],
                                    am[:, :],
                                    start=(half == 0 and qb == 0), stop=False)
                                outh = ppv[:, half, qb * 128:qb * 128 + 8]
                                vhalo = (vhp_sb[0:8, t, h * DH:(h + 1) * DH]
                                         if qb == 0
                                         else vhc[0:8, h * DH:(h + 1) * DH])
                                nc.tensor.matmul(
                                    outh, vhalo, amh[:, :], start=False,
                                    stop=(t == 0 and half == 1 and qb == 1))
                        if t > 0:
                            nc.tensor.matmul(ppv[:, :, :], i96ar_sb[:],
                                             msop[0:96, :, 2 * j:2 * j + 2, :],
                                             start=False, stop=True, perf_mode=DR)
                        nc.vector.tensor_scalar(
                            msoa[0:96, 1, 2 * j:2 * j + 2, :], ppv[:],
                            THA if t == 0 else THA / 2, None, ALU.is_ge)
                        if t < T - 1:
                            nc.scalar.activation(
                                msoa[0:96, 0, 2 * j:2 * j + 2, :], ppv[:],
                                AF.Relu, scale=-0.5,
                                bias=(bcA5 if t == 0 else bcA25)[0:96, :])
                    ms_prev["oa"] = msoa

                    # halo captures for next chunk (after attention reads)
                    nc.gpsimd.tensor_copy(khpv[0:96, :, t, :],
                                          msk[0:96, 1, :, NCH - 8:NCH])
                    nc.sync.dma_start(vhp_sb[0:8, t, :], msv[120:128, 1, 1, :])

                    # ========== proj (ud8 LIF) + x2 = x + o ==========
                    x2 = upool.tile([128, CI6, NCH], BF16, name="x2", tag="x2")
                    for jj in range(3):
                        pf = ps_f.tile([128, 2, NCH], F32, name="psf", tag="psf")
                        for half in range(2):
                            i = 2 * jj + half
                            out = pf[:, half, :]
                            for hp in range(4):
                                nc.tensor.matmul(
                                    out,
                                    wp8_sb[:, 2 * hp:2 * hp + 2,
                                           i * 128:(i + 1) * 128],
                                    msoa[0:96, 1, 2 * hp:2 * hp + 2, :],
                                    start=(half == 0 and hp == 0), stop=False,
                                    perf_mode=DR)
                            nc.tensor.matmul(
                                out, bpp[0:1, :, i * 128:(i + 1) * 128],
                                ones2[0:1, :, :], start=False,
                                stop=(t == 0 and half == 1), perf_mode=DR)
                        if t > 0:
                            nc.tensor.matmul(
                                pf[:, :, :], i128r_sb[:],
                                mso[:, :, 2 * jj:2 * jj + 2, :],
                                start=False, stop=True, perf_mode=DR)
                        nc.vector.tensor_scalar(
                            mso[:, 1, 2 * jj:2 * jj + 2, :], pf[:],
                            TH if t == 0 else TH / 2, None, ALU.is_ge)
                        if t < T - 1:
                            nc.scalar.activation(
                                mso[:, 0, 2 * jj:2 * jj + 2, :], pf[:],
                                AF.Relu, scale=-0.5,
                                bias=bc64 if t == 0 else bc32)
                    nc.gpsimd.tensor_tensor(x2[:], mso[:, 1, :, :],
                                            xbfc[:, :, xcols], ALU.add)
                    x28 = upool.tile([128, CI6, NCH], FP8, name="x28", tag="x28")
                    nc.gpsimd.tensor_copy(x28[:], x2[:])

                    # ========== f1 (staged: evac->SBUF, s/m on Pool) ==========
                    hmp = ms_prev.get("h")
                    u1 = upool.tile([128, M24, NCH], BF16, name="u1", tag="u1")
                    for jj in range(12):
                        pf = ps_f.tile([128, 2, NCH], F32, name="psf", tag="psf")
                        for half in range(2):
                            i = 2 * jj + half
                            out = pf[:, half, :]
                            for p in range(3):
                                nc.tensor.matmul(
                                    out,
                                    w18_sb[:, 2 * p:2 * p + 2,
                                           i * 128:(i + 1) * 128],
                                    x28[:, 2 * p:2 * p + 2, :],
                                    start=(half == 0 and p == 0), stop=False,
                                    perf_mode=DR)
                            nc.tensor.matmul(
                                out, b1p[0:1, :, i * 128:(i + 1) * 128],
                                ones2[0:1, :, :], start=False,
                                stop=(t == 0 and half == 1), perf_mode=DR)
                        if t > 0:
                            nc.tensor.matmul(
                                pf[:, :, :], i128s_sb[:],
                                hmp[:, :, 2 * jj:2 * jj + 2, :],
                                start=False, stop=True, perf_mode=DR)
                        nc.scalar.activation(u1[:, 2 * jj:2 * jj + 2, :], pf[:],
                                             AF.Identity)
                    for g in range(3):
                        gsl = slice(8 * g, 8 * g + 8)
                        nc.gpsimd.tensor_scalar(hm[:, 1, gsl, :], u1[:, gsl, :],
                                                TH, None, ALU.is_ge)
                        if t < T - 1:
                            nc.gpsimd.tensor_scalar(hm[:, 0, gsl, :],
                                                    u1[:, gsl, :], TH, 0.5,
                                                    ALU.min, ALU.mult)
                    ms_prev["h"] = hm

                    # ========== f2 (ud8 LIF) + out = x2 + m ==========
                    of = opool.tile([128, CI6, NCH], F32, name="of", tag="of")
                    for jj in range(3):
                        pf = ps_f.tile([128, 2, NCH], F32, name="psf", tag="psf")
                        for half in range(2):
                            i = 2 * jj + half
                            out = pf[:, half, :]
                            for p in range(12):
                                nc.tensor.matmul(
                                    out,
                                    w28_sb[:, 2 * p:2 * p + 2,
                                           i * 128:(i + 1) * 128],
                                    hm[:, 1, 2 * p:2 * p + 2, :],
                                    start=(half == 0 and p == 0), stop=False,
                                    perf_mode=DR)
                            nc.tensor.matmul(
                                out, b2p[0:1, :, i * 128:(i + 1) * 128],
                                ones2[0:1, :, :], start=False,
                                stop=(t == 0 and half == 1), perf_mode=DR)
                        if t > 0:
                            nc.tensor.matmul(
                                pf[:, :, :], i128r_sb[:],
                                msm[:, :, 2 * jj:2 * jj + 2, :],
                                start=False, stop=True, perf_mode=DR)
                        nc.vector.tensor_scalar(
                            msm[:, 1, 2 * jj:2 * jj + 2, :], pf[:],
                            TH if t == 0 else TH / 2, None, ALU.is_ge)
                        if t < T - 1:
                            nc.scalar.activation(
                                msm[:, 0, 2 * jj:2 * jj + 2, :], pf[:],
                                AF.Relu, scale=-0.5,
                                bias=bc64 if t == 0 else bc32)
                    nc.gpsimd.tensor_tensor(of[:], msm[:, 1, :, :], x2[:], ALU.add)
                    nc.sync.dma_start(
                        out_d[:, c, t].rearrange("a p n -> p a n"), of[:])

    nc.compile()
    return nc


# ---------------- host-side preparation ----------------

def _lhsT(w, s, nci, npart=128):
    """fold BN scale, scale by SW, fp8, and lay out as [npart, nci, out]"""
    wf = (w * s[:, None]).astype(np.float32) * SW
    out_dim = wf.shape[0]
    return np.ascontiguousarray(
        wf.T.reshape(nci, npart, out_dim).transpose(1, 0, 2)).astype(E4)


def _bias_pair(b):
    """double-fp8 (hi, lo) pair of a bias row, as [1, 2*dim]"""
    bf = np.asarray(b, np.float32)
    hi = bf.astype(E4)
    lo = (bf - hi.astype(np.float32)).astype(E4)
    return np.concatenate([hi[None, :], lo[None, :]], axis=0).reshape(1, -1)


def _prep_shared(qw, qb, qs, qt, kw, kb, ks, kt, vw, vb, vs, vt,
                 pw, pb, ps, pt, f1w, f1b, f1s, f1t, f2w, f2b, f2s, f2t):
    out = {}
    out["wq8"] = _lhsT(qw, qs, CI6).reshape(128, CI6 * C)
    out["wk8"] = _lhsT(kw, ks, CI6).reshape(128, CI6 * C)
    out["wv8"] = _lhsT(vw, vs, CI6).reshape(128, CI6 * C)
    out["wp8"] = _lhsT(pw, ps, NH, 96).reshape(96, NH * C)
    out["w18"] = _lhsT(f1w, f1s, CI6).reshape(128, CI6 * HD)
    out["w28"] = _lhsT(f2w, f2s, M24).reshape(128, M24 * C)
    # biases, reordered to match each GEMM's output tiling
    bq = (qb * qs + qt).astype(np.float32) * SW          # by head already natural
    out["bqp"] = _bias_pair(bq)
    bk = (kb * ks + kt).astype(np.float32) * SW
    out["bkp"] = _bias_pair(bk)
    bv = (vb * vs + vt).astype(np.float32) * SW
    out["bvp"] = _bias_pair(bv)
    bp_ = (pb * ps + pt).astype(np.float32) * SW
    out["bpp"] = _bias_pair(bp_)
    b1 = (f1b * f1s + f1t).astype(np.float32) * SW
    out["b1p"] = _bias_pair(b1)
    b2 = (f2b * f2s + f2t).astype(np.float32) * SW
    out["b2p"] = _bias_pair(b2)
    out["ones2"] = np.ones((1, 2 * NCH), dtype=E4)

    qp = np.zeros((16, NCH), dtype=np.float32)
    kp = np.zeros((16, NCH), dtype=np.float32)
    for col in range(NCH):
        j = col % 128
        qp[j // W, col] = 1.0
        jwin = j + W
        for g in range(16):
            kp[g, col] = 0.0 if (W * g <= jwin < W * g + 2 * W) else NEG
    out["qp"] = qp.astype(E4)
    out["kp"] = kp.astype(E4)
    khp = np.full((16, W), NEG, dtype=np.float32)
    khp[0, :] = 0.0
    out["khp"] = np.tile(khp, (1, T)).astype(E4)
    out["khf"] = np.full((16, T * W), NEG, dtype=E4)

    eye96 = np.eye(96, dtype=np.float32)
    out["i96r"] = np.concatenate([-eye96[:, None, :], -(TH / 2) * eye96[:, None, :]],
                                 axis=1).reshape(96, 2 * 96).astype(E4)
    out["i96ar"] = np.concatenate([-eye96[:, None, :], -0.5 * eye96[:, None, :]],
                                  axis=1).reshape(96, 2 * 96).astype(E4)
    eye128 = np.eye(128, dtype=np.float32)
    out["i128s"] = np.concatenate([eye128[:, None, :], -(TH / 2) * eye128[:, None, :]],
                                  axis=1).reshape(128, 2 * 128).astype(E4)
    out["i128r"] = np.concatenate([-eye128[:, None, :], -(TH / 2) * eye128[:, None, :]],
                                  axis=1).reshape(128, 2 * 128).astype(E4)
    out["idT"] = np.eye(128, dtype=E4)
    return out


def prep_in_maps(inputs):
    x = np.asarray(inputs["x"], dtype=np.float32)
    shared = _prep_shared(**{k: np.asarray(v, np.float32)
                             for k, v in inputs.items() if k != "x"})
    in_maps = []
    for b in range(B):
        xb = x[:, b]                                    # [T, N, C]
        y = np.ascontiguousarray(xb.transpose(2, 0, 1)) # [C, T, N]
        y = y.reshape(CI6, 128, T, NCHUNK, NCH)
        arr = np.ascontiguousarray(y.transpose(1, 0, 3, 2, 4)).reshape(128, CI6, TOK)
        m = dict(shared)
        m["xbf"] = arr.astype(BF)
        m["x8"] = arr.astype(E4)
        in_maps.append(m)
    return in_maps


_NC_CACHE = {}


def get_nc():
    if "nc" not in _NC_CACHE:
        _NC_CACHE["nc"] = build_nc()
    return _NC_CACHE["nc"]


def assemble_output(results):
    out = np.empty((T, B, NSEQ, C), dtype=np.float32)
    for b in range(B):
        arr = results[b]["outT"]                        # [CI6, NCHUNK, T, 128, NCH]
        out[:, b] = arr.transpose(2, 1, 4, 0, 3).reshape(T, NSEQ, C)
    return out


def kernel(**inputs):
    nc = get_nc()
    in_maps = prep_in_maps(inputs)
    res = run_bass_kernel_spmd(nc, in_maps, list(range(B)))
    return assemble_output(res.results)


if __name__ == "__main__":
    nc = get_nc()
    print("compiled OK")


# revision 46
# speedup vs baseline: 1.7426x; 1.0543x over previous
"""Trainium2 Bass kernel v3 for nn_Block_59433757442280 (spiking local-attention block).

Data-parallel over B=8 (one batch element per core). Single fused pass,
t-interleaved. v3 restructure vs v2:
- No u-evacuation anywhere: spike (s) and decay (m) passes read GEMM PSUM
  directly (DVE/GpSimd tensor_scalar with const scalars).
- Biases enter PSUM via double-fp8 DR matmuls (hi+lo fp8 pair, ~12-bit
  accurate) instead of Act-evac bias / bf16 ones-matmuls.
- proj/f2 LIF use ud8 form: ud = u*[u<TH] in one scalar_tensor_tensor from
  PSUM; feedback matmul uses (ud, zeros) DR pair with [0.5I; 0].
- proj spike fused with residual: x2 = (pf >= TH) + x via stt.
  f2 spike fused with output: out = (pf >= TH) + x2 via stt.
- Attention: two q-blocks share one PSUM bank; one exp per head (no accum),
  row-sums via DVE tensor_reduce; halo transpose/PV restricted to the 8
  first-window columns.
"""

import sys

for _p in ("/opt/trn_rl_repo",):
    if _p not in sys.path:
        sys.path.insert(0, _p)

import numpy as np
import ml_dtypes

import concourse.bass as bass
import concourse.tile as tile
from concourse import mybir, bacc
from concourse.bass_utils import run_bass_kernel_spmd

F32 = mybir.dt.float32
BF16 = mybir.dt.bfloat16
FP8 = mybir.dt.float8e4
AF = mybir.ActivationFunctionType
ALU = mybir.AluOpType
DR = mybir.MatmulPerfMode.DoubleRow
BF = ml_dtypes.bfloat16
E4 = ml_dtypes.float8_e4m3

# problem constants
T, B, NSEQ, C, HD = 4, 8, 1024, 768, 3072
NH, DH, W = 8, 96, 8
TOK = T * NSEQ
CI6 = C // 128            # 6 input-channel tiles
M24 = HD // 128           # 24 f1 output tiles
NCH = 256                 # seq positions per chunk
NCHUNK = NSEQ // NCH      # 4
PB = NCH // 128           # 2 position blocks per chunk
SCALE = float(DH) ** -0.5
SW = 64.0                 # weight scale (power of 2)
TH = 2.0 * SW             # LIF threshold in u domain (vth=1)
THA = 1.0                 # attn-lif threshold (vth=0.5, unscaled)
NEG = -240.0              # fp8 mask value


def build_nc():
    nc = bacc.Bacc(None, target_bir_lowering=False, debug=False)

    # ---- DRAM inputs (per core) ----
    x8_d = nc.dram_tensor("x8", [128, CI6, TOK], FP8, kind="ExternalInput")
    xbf_d = nc.dram_tensor("xbf", [128, CI6, TOK], BF16, kind="ExternalInput")
    wq8_d = nc.dram_tensor("wq8", [128, CI6 * C], FP8, kind="ExternalInput")
    wk8_d = nc.dram_tensor("wk8", [128, CI6 * C], FP8, kind="ExternalInput")
    wv8_d = nc.dram_tensor("wv8", [128, CI6 * C], FP8, kind="ExternalInput")
    wp8_d = nc.dram_tensor("wp8", [96, NH * C], FP8, kind="ExternalInput")
    w18_d = nc.dram_tensor("w18", [128, CI6 * HD], FP8, kind="ExternalInput")
    w28_d = nc.dram_tensor("w28", [128, M24 * C], FP8, kind="ExternalInput")
    # double-fp8 bias pairs
    bqp_d = nc.dram_tensor("bqp", [1, 2 * C], FP8, kind="ExternalInput")
    bkp_d = nc.dram_tensor("bkp", [1, 2 * C], FP8, kind="ExternalInput")
    bvp_d = nc.dram_tensor("bvp", [1, 2 * C], FP8, kind="ExternalInput")
    bpp_d = nc.dram_tensor("bpp", [1, 2 * C], FP8, kind="ExternalInput")
    b1p_d = nc.dram_tensor("b1p", [1, 2 * HD], FP8, kind="ExternalInput")
    b2p_d = nc.dram_tensor("b2p", [1, 2 * C], FP8, kind="ExternalInput")
    ones2_d = nc.dram_tensor("ones2", [1, 2 * NCH], FP8, kind="ExternalInput")
    qp_d = nc.dram_tensor("qp", [16, NCH], FP8, kind="ExternalInput")
    kp_d = nc.dram_tensor("kp", [16, NCH], FP8, kind="ExternalInput")
    khp_d = nc.dram_tensor("khp", [16, T * W], FP8, kind="ExternalInput")
    khf_d = nc.dram_tensor("khf", [16, T * W], FP8, kind="ExternalInput")
    i96r_d = nc.dram_tensor("i96r", [96, 2 * 96], FP8, kind="ExternalInput")
    i96ar_d = nc.dram_tensor("i96ar", [96, 2 * 96], FP8, kind="ExternalInput")
    i128r_d = nc.dram_tensor("i128r", [128, 2 * 128], FP8, kind="ExternalInput")
    i128s_d = nc.dram_tensor("i128s", [128, 2 * 128], FP8, kind="ExternalInput")
    idT_d = nc.dram_tensor("idT", [128, 128], FP8, kind="ExternalInput")
    out_d = nc.dram_tensor("outT", [CI6, NCHUNK, T, 128, NCH], F32,
                           kind="ExternalOutput")

    with tile.TileContext(nc) as tc:
        from contextlib import ExitStack
        with ExitStack() as top:
            cpool = top.enter_context(tc.tile_pool(name="const", bufs=1))
            mspool = top.enter_context(tc.tile_pool(name="ms", bufs=1))
            upool = top.enter_context(tc.tile_pool(name="u", bufs=1))
            xpool = top.enter_context(tc.tile_pool(name="x", bufs=2))
            apool = top.enter_context(tc.tile_pool(name="attn", bufs=3))
            opool = top.enter_context(tc.tile_pool(name="of", bufs=2))
            ps_qk = top.enter_context(tc.tile_pool(name="psqk", bufs=2, space="PSUM"))
            ps_v = top.enter_context(tc.tile_pool(name="psv", bufs=1, space="PSUM"))
            ps_sim = top.enter_context(tc.tile_pool(name="pssim", bufs=1, space="PSUM"))
            ps_tp = top.enter_context(tc.tile_pool(name="pstp", bufs=1, space="PSUM"))
            ps_pv = top.enter_context(tc.tile_pool(name="pspv", bufs=1, space="PSUM"))
            ps_f = top.enter_context(tc.tile_pool(name="psf", bufs=2, space="PSUM"))

            # ---- persistent SBUF ----
            wq8_sb = cpool.tile([128, CI6, C], FP8, name="wq8", tag="wq8")
            nc.sync.dma_start(wq8_sb[:], wq8_d.rearrange("p (a b) -> p a b", a=CI6))
            wk8_sb = cpool.tile([128, CI6, C], FP8, name="wk8", tag="wk8")
            nc.gpsimd.dma_start(wk8_sb[:], wk8_d.rearrange("p (a b) -> p a b", a=CI6))
            wv8_sb = cpool.tile([128, CI6, C], FP8, name="wv8", tag="wv8")
            nc.gpsimd.dma_start(wv8_sb[:], wv8_d.rearrange("p (a b) -> p a b", a=CI6))
            wp8_sb = cpool.tile([96, NH, C], FP8, name="wp8", tag="wp8")
            nc.scalar.dma_start(wp8_sb[:], wp8_d.rearrange("p (a b) -> p a b", a=NH))
            w18_sb = cpool.tile([128, CI6, HD], FP8, name="w18", tag="w18")
            nc.scalar.dma_start(w18_sb[:], w18_d.rearrange("p (a b) -> p a b", a=CI6))
            w28_sb = cpool.tile([128, M24, C], FP8, name="w28", tag="w28")
            nc.gpsimd.dma_start(w28_sb[:], w28_d.rearrange("p (a b) -> p a b", a=M24))
            bqp = cpool.tile([1, 2, C], FP8, name="bqp", tag="bqp")
            nc.sync.dma_start(bqp[:], bqp_d.rearrange("p (a b) -> p a b", a=2))
            bkp = cpool.tile([1, 2, C], FP8, name="bkp", tag="bkp")
            nc.sync.dma_start(bkp[:], bkp_d.rearrange("p (a b) -> p a b", a=2))
            bvp = cpool.tile([1, 2, C], FP8, name="bvp", tag="bvp")
            nc.sync.dma_start(bvp[:], bvp_d.rearrange("p (a b) -> p a b", a=2))
            bpp = cpool.tile([1, 2, C], FP8, name="bpp", tag="bpp")
            nc.sync.dma_start(bpp[:], bpp_d.rearrange("p (a b) -> p a b", a=2))
            b1p = cpool.tile([1, 2, HD], FP8, name="b1p", tag="b1p")
            nc.sync.dma_start(b1p[:], b1p_d.rearrange("p (a b) -> p a b", a=2))
            b2p = cpool.tile([1, 2, C], FP8, name="b2p", tag="b2p")
            nc.sync.dma_start(b2p[:], b2p_d.rearrange("p (a b) -> p a b", a=2))
            ones2 = cpool.tile([1, 2, NCH], FP8, name="ones2", tag="ones2")
            nc.sync.dma_start(ones2[:], ones2_d.rearrange("p (a b) -> p a b", a=2))
            onesP = cpool.tile([1, 2, 128], FP8, name="onesP", tag="onesP")
            nc.sync.dma_start(onesP[:], ones2_d.rearrange("p (a b) -> p a b",
                                                          a=2)[:, :, 0:128])
            i96r_sb = cpool.tile([96, 2, 96], FP8, name="i96r", tag="i96r")
            nc.sync.dma_start(i96r_sb[:], i96r_d.rearrange("p (a b) -> p a b", a=2))
            i96ar_sb = cpool.tile([96, 2, 96], FP8, name="i96ar", tag="i96ar")
            nc.sync.dma_start(i96ar_sb[:], i96ar_d.rearrange("p (a b) -> p a b", a=2))
            i128r_sb = cpool.tile([128, 2, 128], FP8, name="i128r", tag="i128r")
            nc.sync.dma_start(i128r_sb[:], i128r_d.rearrange("p (a b) -> p a b", a=2))
            # Act bias const tiles for m~ = relu(thr/2 - 0.5*psum)
            bc64 = cpool.tile([128, 1], F32, name="bc64", tag="bc64")
            nc.vector.memset(bc64[:], 64.0)
            bc32 = cpool.tile([128, 1], F32, name="bc32", tag="bc32")
            nc.vector.memset(bc32[:], 32.0)
            bcA5 = cpool.tile([128, 1], F32, name="bcA5", tag="bcA5")
            nc.vector.memset(bcA5[:], 0.5)
            bcA25 = cpool.tile([128, 1], F32, name="bcA25", tag="bcA25")
            nc.vector.memset(bcA25[:], 0.25)
            i128s_sb = cpool.tile([128, 2, 128], FP8, name="i128s", tag="i128s")
            nc.sync.dma_start(i128s_sb[:], i128s_d.rearrange("p (a b) -> p a b", a=2))
            idT_sb = cpool.tile([128, 128], FP8, name="idT", tag="idT")
            nc.sync.dma_start(idT_sb[:], idT_d[:])

            # k-halo tiles: [112, NH, T, W]; pattern rows 96:112 loaded once
            khc = cpool.tile([112, NH, T, W], FP8, name="khc", tag="khc")
            khpv = cpool.tile([112, NH, T, W], FP8, name="khpv", tag="khpv")
            khf = cpool.tile([112, NH, T, W], FP8, name="khf", tag="khf")
            for h in range(NH):
                nc.gpsimd.dma_start(khc[96:112, h, :, :],
                                    khp_d.rearrange("g (t w) -> g t w", t=T))
                nc.gpsimd.dma_start(khpv[96:112, h, :, :],
                                    khp_d.rearrange("g (t w) -> g t w", t=T))
                nc.gpsimd.dma_start(khf[96:112, h, :, :],
                                    khf_d.rearrange("g (t w) -> g t w", t=T))
            nc.vector.memset(khf[0:96, :, :, :], 0.0)
            nc.vector.memset(khpv[0:96, :, :, :], 0.0)
            # v-halo: prev-chunk [8, T, C]; cur [8, C] rotating
            vhp_sb = cpool.tile([8, T, C], FP8, name="vhp", tag="vhp")
            nc.vector.memset(vhp_sb[:], 0.0)

            msqk_t = {}
            for nm in ("q", "k"):
                msc = mspool.tile([112, 2, NH, NCH], FP8, name=f"ms{nm}",
                                  tag=f"ms{nm}")
                pat = qp_d if nm == "q" else kp_d
                for h in range(NH):
                    nc.gpsimd.dma_start(msc[96:112, 1, h, :], pat[:])
                msqk_t[nm] = msc

            # persistent transpose PSUM bank; halo rows 8:128 stay zero
            tpp = ps_tp.tile([128, 288], FP8, name="tpp", tag="tp")
            z8 = cpool.tile([8, 128], FP8, name="z8", tag="z8")
            nc.vector.memset(z8[:], 0.0)
            nc.tensor.matmul(tpp[:, 256:272:2], z8[:], idT_sb[0:8, 0:8],
                             start=True, stop=True, is_transpose=True)
            # persistent m/s tiles (rewritten each t; WAR deps serialize)
            msv = mspool.tile([128, 2, PB, C], FP8, name="msv", tag="msv")
            msoa = mspool.tile([96, 2, NH, NCH], FP8, name="msoa", tag="msoa")
            hm = mspool.tile([128, 2, M24, NCH], FP8, name="hm", tag="hm")
            mso = mspool.tile([128, 2, CI6, NCH], FP8, name="mso", tag="mso")
            msm = mspool.tile([128, 2, CI6, NCH], FP8, name="msm", tag="msm")

            ms_prev = {}

            for c in range(NCHUNK):
                x8c = xpool.tile([128, CI6, NSEQ], FP8, name="x8c", tag="x8c")
                nc.sync.dma_start(x8c[:], x8_d[:, :, c * NSEQ:(c + 1) * NSEQ])
                xbfc = xpool.tile([128, CI6, NSEQ], BF16, name="xbfc", tag="xbfc")
                nc.scalar.dma_start(xbfc[:], xbf_d[:, :, c * NSEQ:(c + 1) * NSEQ])
                for t in range(T):
                    col0 = t * NCH
                    xcols = slice(col0, col0 + NCH)

                    # ========== q, k GEMMs + LIF ==========
                    for nm, w8, bp in (("q", wq8_sb, bqp), ("k", wk8_sb, bkp)):
                        msc = msqk_t[nm]
                        msp = ms_prev.get(nm)
                        for j in range(4):
                            ps = ps_qk.tile([96, 2, NCH], F32, name="psqk",
                                            tag="psqk")
                            for half in range(2):
                                h = 2 * j + half
                                out = ps[:, half, :]
                                for p in range(3):
                                    nc.tensor.matmul(
                                        out,
                                        w8[:, 2 * p:2 * p + 2, h * DH:(h + 1) * DH],
                                        x8c[:, 2 * p:2 * p + 2, xcols],
                                        start=(half == 0 and p == 0), stop=False,
                                        perf_mode=DR)
                                nc.tensor.matmul(
                                    out, bp[0:1, :, h * DH:(h + 1) * DH],
                                    ones2[0:1, :, :], start=False,
                                    stop=(t == 0 and half == 1), perf_mode=DR)
                            if t > 0:
                                nc.tensor.matmul(
                                    ps[:, :, :], i96r_sb[:],
                                    msp[0:96, :, 2 * j:2 * j + 2, :],
                                    start=False, stop=True, perf_mode=DR)
                            nc.vector.tensor_scalar(
                                msc[0:96, 1, 2 * j:2 * j + 2, :], ps[:],
                                TH if t == 0 else TH / 2, None, ALU.is_ge)
                            if t < T - 1:
                                nc.scalar.activation(
                                    msc[0:96, 0, 2 * j:2 * j + 2, :], ps[:],
                                    AF.Relu, scale=-0.5,
                                    bias=(bc64 if t == 0 else bc32)[0:96, :])
                        ms_prev[nm] = msc
                    msq, msk = msqk_t["q"], msqk_t["k"]

                    # k halos: within-chunk (cols 120:128) for qb=1
                    nc.gpsimd.tensor_copy(khc[0:96, :, t, :],
                                          msk[0:96, 1, :, 120:128])

                    # ========== v GEMM + LIF ==========
                    mspv = ms_prev.get("v")
                    for pb in range(PB):
                        pcol = col0 + pb * 128
                        for half in range(2):
                            ps = ps_v.tile([128, 384], F32, name="psv", tag="psv")
                            for p in range(3):
                                nc.tensor.matmul(
                                    ps[:], x8c[:, 2 * p:2 * p + 2, pcol:pcol + 128],
                                    wv8_sb[:, 2 * p:2 * p + 2,
                                           half * 384:(half + 1) * 384],
                                    start=(p == 0), stop=False, perf_mode=DR)
                            nc.tensor.matmul(
                                ps[:], onesP[0:1, :, :],
                                bvp[0:1, :, half * 384:(half + 1) * 384],
                                start=False, stop=(t == 0), perf_mode=DR)
                            if t > 0:
                                nc.tensor.matmul(
                                    ps[:], i128r_sb[:],
                                    mspv[:, :, pb, half * 384:(half + 1) * 384],
                                    start=False, stop=True, perf_mode=DR)
                            nc.vector.tensor_scalar(
                                msv[:, 1, pb, half * 384:(half + 1) * 384], ps[:],
                                TH if t == 0 else TH / 2, None, ALU.is_ge)
                            if t < T - 1:
                                nc.scalar.activation(
                                    msv[:, 0, pb, half * 384:(half + 1) * 384],
                                    ps[:], AF.Relu, scale=-0.5,
                                    bias=bc64 if t == 0 else bc32)
                    ms_prev["v"] = msv
                    # v halo for within-chunk qb=1 (pb0 tail)
                    vhc = apool.tile([8, C], FP8, name="vhc", tag="vhc")
                    nc.sync.dma_start(vhc[:], msv[120:128, 1, 0, :])

                    # ========== attention ==========
                    msop = ms_prev.get("oa")
                    for j in range(4):
                        ppv = ps_pv.tile([96, 2, NCH], F32, name="pspv", tag="pspv")
                        for half in range(2):
                            h = 2 * j + half
                            psm = ps_sim.tile([128, 2, 136], F32, name="pssim",
                                              tag="pssim")
                            for qb in range(2):
                                qsl = msq[0:112, 1, h, qb * 128:(qb + 1) * 128]
                                nc.tensor.matmul(
                                    psm[:, qb, 0:128], qsl,
                                    msk[0:112, 1, h, qb * 128:(qb + 1) * 128],
                                    start=(qb == 0), stop=False)
                                halo = (khf if (c == 0 and qb == 0)
                                        else khpv if qb == 0 else khc)
                                nc.tensor.matmul(psm[:, qb, 128:136], qsl,
                                                 halo[0:112, h, t, :],
                                                 start=False, stop=(qb == 1))
                            attn = apool.tile([128, 2, 136], BF16, name="attn",
                                              tag="attn")
                            nc.scalar.activation(attn[:], psm[:], AF.Exp,
                                                 scale=SCALE)
                            rs = apool.tile([128, 2], BF16, name="rs", tag="rs")
                            with nc.allow_low_precision(reason="softmax rowsum"):
                                nc.vector.tensor_reduce(rs[:], attn[:],
                                                        mybir.AxisListType.X,
                                                        ALU.add)
                            rc = apool.tile([128, 2], F32, name="rc", tag="rc")
                            nc.vector.reciprocal(rc[:], rs[:])
                            for qb in range(2):
                                at8 = apool.tile([128, 136], FP8, name="at8",
                                                 tag="at8")
                                nc.vector.tensor_scalar(at8[:], attn[:, qb, :],
                                                        rc[:, qb:qb + 1], None,
                                                        ALU.mult)
                                nc.tensor.matmul(tpp[:, 0:256:2], at8[:, 0:128],
                                                 idT_sb[:], start=True, stop=True,
                                                 is_transpose=True)
                                nc.tensor.matmul(tpp[0:8, 256:272:2],
                                                 at8[0:8, 128:136],
                                                 idT_sb[0:8, 0:8], start=False,
                                                 stop=False, is_transpose=True,
                                                 skip_group_check=True)
                                am = apool.tile([128, 136], FP8, name="am", tag="am")
                                nc.vector.tensor_copy(am[:, :], tpp[:, 0:272:2])
                                out = ppv[:, half, qb * 128:(qb + 1) * 128]
                                nc.tensor.matmul(
                                    out, msv[:, 1, qb, h * DH:(h + 1) * DH],
                                    am[:, 0:128],
                                    start=(half == 0 and qb == 0), stop=False)
                                outh = ppv[:, half, qb * 128:qb * 128 + 8]
                                vhalo = (vhp_sb[0:8, t, h * DH:(h + 1) * DH]
                                         if qb == 0
                                         else vhc[0:8, h * DH:(h + 1) * DH])
                                nc.tensor.matmul(
                                    outh, vhalo, am[0:8, 128:136], start=False,
                                    stop=(t == 0 and half == 1 and qb == 1))
                        if t > 0:
                            nc.tensor.matmul(ppv[:, :, :], i96ar_sb[:],
                                             msop[0:96, :, 2 * j:2 * j + 2, :],
                                             start=False, stop=True, perf_mode=DR)
                        nc.vector.tensor_scalar(
                            msoa[0:96, 1, 2 * j:2 * j + 2, :], ppv[:],
                            THA if t == 0 else THA / 2, None, ALU.is_ge)
                        if t < T - 1:
                            nc.scalar.activation(
                                msoa[0:96, 0, 2 * j:2 * j + 2, :], ppv[:],
                                AF.Relu, scale=-0.5,
                                bias=(bcA5 if t == 0 else bcA25)[0:96, :])
                    ms_prev["oa"] = msoa

                    # halo captures for next chunk (after attention reads)
                    nc.gpsimd.tensor_copy(khpv[0:96, :, t, :],
                                          msk[0:96, 1, :, NCH - 8:NCH])
                    nc.sync.dma_start(vhp_sb[0:8, t, :], msv[120:128, 1, 1, :])

                    # ========== proj (ud8 LIF) + x2 = x + o ==========
                    x2 = upool.tile([128, CI6, NCH], BF16, name="x2", tag="x2")
                    for jj in range(3):
                        pf = ps_f.tile([128, 2, NCH], F32, name="psf", tag="psf")
                        for half in range(2):
                            i = 2 * jj + half
                            out = pf[:, half, :]
                            for hp in range(4):
                                nc.tensor.matmul(
                                    out,
                                    wp8_sb[:, 2 * hp:2 * hp + 2,
                                           i * 128:(i + 1) * 128],
                                    msoa[0:96, 1, 2 * hp:2 * hp + 2, :],
                                    start=(half == 0 and hp == 0), stop=False,
                                    perf_mode=DR)
                            nc.tensor.matmul(
                                out, bpp[0:1, :, i * 128:(i + 1) * 128],
                                ones2[0:1, :, :], start=False,
                                stop=(t == 0 and half == 1), perf_mode=DR)
                        if t > 0:
                            nc.tensor.matmul(
                                pf[:, :, :], i128r_sb[:],
                                mso[:, :, 2 * jj:2 * jj + 2, :],
                                start=False, stop=True, perf_mode=DR)
                        nc.vector.tensor_scalar(
                            mso[:, 1, 2 * jj:2 * jj + 2, :], pf[:],
                            TH if t == 0 else TH / 2, None, ALU.is_ge)
                        if t < T - 1:
                            nc.scalar.activation(
                                mso[:, 0, 2 * jj:2 * jj + 2, :], pf[:],
                                AF.Relu, scale=-0.5,
                                bias=bc64 if t == 0 else bc32)
                    nc.gpsimd.tensor_tensor(x2[:], mso[:, 1, :, :],
                                            xbfc[:, :, xcols], ALU.add)
                    x28 = upool.tile([128, CI6, NCH], FP8, name="x28", tag="x28")
                    nc.gpsimd.tensor_copy(x28[:], x2[:])

                    # ========== f1 (staged: evac->SBUF, s/m on Pool) ==========
                    hmp = ms_prev.get("h")
                    u1 = upool.tile([128, M24, NCH], BF16, name="u1", tag="u1")
                    for jj in range(12):
                        pf = ps_f.tile([128, 2, NCH], F32, name="psf", tag="psf")
                        for half in range(2):
                            i = 2 * jj + half
                            out = pf[:, half, :]
                            for p in range(3):
                                nc.tensor.matmul(
                                    out,
                                    w18_sb[:, 2 * p:2 * p + 2,
                                           i * 128:(i + 1) * 128],
                                    x28[:, 2 * p:2 * p + 2, :],
                                    start=(half == 0 and p == 0), stop=False,
                                    perf_mode=DR)
                            nc.tensor.matmul(
                                out, b1p[0:1, :, i * 128:(i + 1) * 128],
                                ones2[0:1, :, :], start=False,
                                stop=(t == 0 and half == 1), perf_mode=DR)
                        if t > 0:
                            nc.tensor.matmul(
                                pf[:, :, :], i128s_sb[:],
                                hmp[:, :, 2 * jj:2 * jj + 2, :],
                                start=False, stop=True, perf_mode=DR)
                        nc.scalar.activation(u1[:, 2 * jj:2 * jj + 2, :], pf[:],
                                             AF.Identity)
                    for g in range(3):
                        gsl = slice(8 * g, 8 * g + 8)
                        nc.gpsimd.tensor_scalar(hm[:, 1, gsl, :], u1[:, gsl, :],
                                                TH, None, ALU.is_ge)
                        if t < T - 1:
                            nc.gpsimd.tensor_scalar(hm[:, 0, gsl, :],
                                                    u1[:, gsl, :], TH, 0.5,
                                                    ALU.min, ALU.mult)
                    ms_prev["h"] = hm

                    # ========== f2 (ud8 LIF) + out = x2 + m ==========
                    of = opool.tile([128, CI6, NCH], F32, name="of", tag="of")
                    for jj in range(3):
                        pf = ps_f.tile([128, 2, NCH], F32, name="psf", tag="psf")
                        for half in range(2):
                            i = 2 * jj + half
                            out = pf[:, half, :]
                            for p in range(12):
                                nc.tensor.matmul(
                                    out,
                                    w28_sb[:, 2 * p:2 * p + 2,
                                           i * 128:(i + 1) * 128],
                                    hm[:, 1, 2 * p:2 * p + 2, :],
                                    start=(half == 0 and p == 0), stop=False,
                                    perf_mode=DR)
                            nc.tensor.matmul(
                                out, b2p[0:1, :, i * 128:(i + 1) * 128],
                                ones2[0:1, :, :], start=False,
                                stop=(t == 0 and half == 1), perf_mode=DR)
                        if t > 0:
                            nc.tensor.matmul(
                                pf[:, :, :], i128r_sb[:],
                                msm[:, :, 2 * jj:2 * jj + 2, :],
                                start=False, stop=True, perf_mode=DR)
                        nc.vector.tensor_scalar(
                            msm[:, 1, 2 * jj:2 * jj + 2, :], pf[:],
                            TH if t == 0 else TH / 2, None, ALU.is_ge)
                        if t < T - 1:
                            nc.scalar.activation(
                                msm[:, 0, 2 * jj:2 * jj + 2, :], pf[:],
                                AF.Relu, scale=-0.5,
                                bias=bc64 if t == 0 else bc32)
                    nc.gpsimd.tensor_tensor(of[:], msm[:, 1, :, :], x2[:], ALU.add)
                    nc.sync.dma_start(
                        out_d[:, c, t].rearrange("a p n -> p a n"), of[:])

    nc.compile()
    return nc


# ---------------- host-side preparation ----------------

def _lhsT(w, s, nci, npart=128):
    """fold BN scale, scale by SW, fp8, and lay out as [npart, nci, out]"""
    wf = (w * s[:, None]).astype(np.float32) * SW
    out_dim = wf.shape[0]
    return np.ascontiguousarray(
        wf.T.reshape(nci, npart, out_dim).transpose(1, 0, 2)).astype(E4)


def _bias_pair(b):
    """double-fp8 (hi, lo) pair of a bias row, as [1, 2*dim]"""
    bf = np.asarray(b, np.float32)
    hi = bf.astype(E4)
    lo = (bf - hi.astype(np.float32)).astype(E4)
    return np.concatenate([hi[None, :], lo[None, :]], axis=0).reshape(1, -1)


def _prep_shared(qw, qb, qs, qt, kw, kb, ks, kt, vw, vb, vs, vt,
                 pw, pb, ps, pt, f1w, f1b, f1s, f1t, f2w, f2b, f2s, f2t):
    out = {}
    out["wq8"] = _lhsT(qw, qs, CI6).reshape(128, CI6 * C)
    out["wk8"] = _lhsT(kw, ks, CI6).reshape(128, CI6 * C)
    out["wv8"] = _lhsT(vw, vs, CI6).reshape(128, CI6 * C)
    out["wp8"] = _lhsT(pw, ps, NH, 96).reshape(96, NH * C)
    out["w18"] = _lhsT(f1w, f1s, CI6).reshape(128, CI6 * HD)
    out["w28"] = _lhsT(f2w, f2s, M24).reshape(128, M24 * C)
    # biases, reordered to match each GEMM's output tiling
    bq = (qb * qs + qt).astype(np.float32) * SW          # by head already natural
    out["bqp"] = _bias_pair(bq)
    bk = (kb * ks + kt).astype(np.float32) * SW
    out["bkp"] = _bias_pair(bk)
    bv = (vb * vs + vt).astype(np.float32) * SW
    out["bvp"] = _bias_pair(bv)
    bp_ = (pb * ps + pt).astype(np.float32) * SW
    out["bpp"] = _bias_pair(bp_)
    b1 = (f1b * f1s + f1t).astype(np.float32) * SW
    out["b1p"] = _bias_pair(b1)
    b2 = (f2b * f2s + f2t).astype(np.float32) * SW
    out["b2p"] = _bias_pair(b2)
    out["ones2"] = np.ones((1, 2 * NCH), dtype=E4)

    qp = np.zeros((16, NCH), dtype=np.float32)
    kp = np.zeros((16, NCH), dtype=np.float32)
    for col in range(NCH):
        j = col % 128
        qp[j // W, col] = 1.0
        jwin = j + W
        for g in range(16):
            kp[g, col] = 0.0 if (W * g <= jwin < W * g + 2 * W) else NEG
    out["qp"] = qp.astype(E4)
    out["kp"] = kp.astype(E4)
    khp = np.full((16, W), NEG, dtype=np.float32)
    khp[0, :] = 0.0
    out["khp"] = np.tile(khp, (1, T)).astype(E4)
    out["khf"] = np.full((16, T * W), NEG, dtype=E4)

    eye96 = np.eye(96, dtype=np.float32)
    out["i96r"] = np.concatenate([-eye96[:, None, :], -(TH / 2) * eye96[:, None, :]],
                                 axis=1).reshape(96, 2 * 96).astype(E4)
    out["i96ar"] = np.concatenate([-eye96[:, None, :], -0.5 * eye96[:, None, :]],
                                  axis=1).reshape(96, 2 * 96).astype(E4)
    eye128 = np.eye(128, dtype=np.float32)
    out["i128s"] = np.concatenate([eye128[:, None, :], -(TH / 2) * eye128[:, None, :]],
                                  axis=1).reshape(128, 2 * 128).astype(E4)
    out["i128r"] = np.concatenate([-eye128[:, None, :], -(TH / 2) * eye128[:, None, :]],
                                  axis=1).reshape(128, 2 * 128).astype(E4)
    out["idT"] = np.eye(128, dtype=E4)
    return out


def prep_in_maps(inputs):
    x = np.asarray(inputs["x"], dtype=np.float32)
    shared = _prep_shared(**{k: np.asarray(v, np.float32)
                             for k, v in inputs.items() if k != "x"})
    in_maps = []
    for b in range(B):
        xb = x[:, b]                                    # [T, N, C]
        y = np.ascontiguousarray(xb.transpose(2, 0, 1)) # [C, T, N]
        y = y.reshape(CI6, 128, T, NCHUNK, NCH)
        arr = np.ascontiguousarray(y.transpose(1, 0, 3, 2, 4)).reshape(128, CI6, TOK)
        m = dict(shared)
        m["xbf"] = arr.astype(BF)
        m["x8"] = arr.astype(E4)
        in_maps.append(m)
    return in_maps


_NC_CACHE = {}


def get_nc():
    if "nc" not in _NC_CACHE:
        _NC_CACHE["nc"] = build_nc()
    return _NC_CACHE["nc"]


def assemble_output(results):
    out = np.empty((T, B, NSEQ, C), dtype=np.float32)
    for b in range(B):
        arr = results[b]["outT"]                        # [CI6, NCHUNK, T, 128, NCH]
        out[:, b] = arr.transpose(2, 1, 4, 0, 3).reshape(T, NSEQ, C)
    return out


def kernel(**inputs):
    nc = get_nc()
    in_maps = prep_in_maps(inputs)
    res = run_bass_kernel_spmd(nc, in_maps, list(range(B)))
    return assemble_output(res.results)


if __name__ == "__main__":
    nc = get_nc()
    print("compiled OK")
